# revision 7
# baseline (speedup 1.0000x reference)
# Triplet-margin loss kernel for Trainium2 (Bass/Tile), batch-sharded
# across 8 NeuronCores.
#
# reference math (torch F.pairwise_distance semantics):
#   d_ap[b,p] = || anc[b] - pos[b,p] + eps ||_2
#   d_an[b,n] = || anc[b] - neg[b,n] + eps ||_2
#   loss = mean_{b,p,n} max(d_ap[b,p] - d_an[b,n] + margin, 0)
#
# v2 design (diff+square, measured op costs on HW):
#   DVE stt diff: out = (x - eps) - a  [fp32,fp32 -> fp16], no accum,
#     cadence ~1135ns on alternating ring buffers (same-buffer WAW
#     back-to-back stalls ~2x, hence the ring).
#   ACT Square(diff fp16) + accum -> d2 column, cadence ~1366ns (ACT has
#     no 16-bit fast path; dtype does not matter for ACT).
#   This removes the baseline's norm-combine (d2c/stt), the ||a'||^2
#   prologue and the sqrt bias coupling: d = sqrt(sum(diff^2)) directly.
#   tensor_tensor_reduce is NOT used: it crashes the exec unit on this
#   HW (NRT_EXEC_UNIT_UNRECOVERABLE) for both fp32 and bf16 operands.
#   Slice order per tile is neg0..neg15 then pos0..pos7 so d_an (sqrt of
#   the 16 neg cols) completes mid-tile and each pos column can be
#   sqrt'ed + paired immediately after its own sum -> ~0.5us tail.
#   Pairing on DVE: accum of min(d_an - d_ap, 1) over the 16 neg cols
#   (in1 = ones tile); host computes hinge = 16 - accum, so no margin
#   tensor and no Relu in the ACT table (table = Square+Sqrt only).
#   The last K_DVE pos sums of tile 1 run on DVE (stt bypass/mult accum,
#   ~1267ns) while ACT drains its queue - balances the engines.
#   Tile-0 pairing is emitted in the middle of tile-1's diff stream (its
#   deps are ready by then; ACT's slack absorbs the 2us of DVE time).
#   anc0 is the FIRST transfer on the sync HWDGE queue (diffs need it);
#   anc1 + both output DMAs ride the idle Tensor engine's HWDGE queue.
#   GpSimd is completely unused (its SWDGE drain made teardown longer).

import numpy as np

import concourse.bacc as bacc
import concourse.mybir as mybir
import concourse.tile as tile
from concourse import bass_utils

B, Z = 2048, 1024
NUM_POS, NUM_NEG = 8, 16
NJ = NUM_POS + NUM_NEG
MARGIN, EPS = 1.0, 1e-6
N_CORES = 8
BL = B // N_CORES  # 256 rows of anc per core
P = 128
NT = BL // P  # 2 batch-tiles per core
XP_BUFS = 15
RING = 12  # fp16 diff ring buffers
K_DVE = 3  # last K pos sums of tile 1 run on DVE
N_SINGLES = 4  # lead single-slice chunks per tile
PAIR_T0_AT = 9  # tile-0 pairing emitted after this many tile-1 diffs

F32 = mybir.dt.float32
FP16 = mybir.dt.float16
AF = mybir.ActivationFunctionType
OP = mybir.AluOpType

# chunk list: (first_slice, n_slices) in the neg-first slice order
CHUNKS = (
    [(j, 1) for j in range(N_SINGLES)]
    + [(j, 2) for j in range(N_SINGLES, NJ, 2)]
)


def _emit(tc, nc, anc, pos, neg, out):
    v = nc.vector
    act = nc.scalar
    pos2 = pos.rearrange("(b j) z -> b (j z)", j=NUM_POS)  # [BL, 8*Z]
    neg2 = neg.rearrange("(b j) z -> b (j z)", j=NUM_NEG)  # [BL, 16*Z]

    def slice_src(t, jj):
        # slice jj of tile t: jj 0..15 -> neg col jj, 16..23 -> pos col jj-16
        b0 = t * P
        if jj < NUM_NEG:
            return neg2[b0 : b0 + P, jj * Z : (jj + 1) * Z]
        return pos2[b0 : b0 + P, (jj - NUM_NEG) * Z : (jj - NUM_NEG + 1) * Z]

    def chunk_src(t, jj0, nsl):
        b0 = t * P
        if jj0 < NUM_NEG:
            assert jj0 + nsl <= NUM_NEG or jj0 >= NUM_NEG
            return neg2[b0 : b0 + P, jj0 * Z : (jj0 + nsl) * Z]
        return pos2[b0 : b0 + P, (jj0 - NUM_NEG) * Z : (jj0 - NUM_NEG + nsl) * Z]

    with (
        tc.tile_pool(name="xp", bufs=XP_BUFS) as xp,
        tc.tile_pool(name="rp", bufs=1) as rp,
        tc.tile_pool(name="ap_", bufs=1) as apool,
        tc.tile_pool(name="sp", bufs=1) as sp,
    ):
        ring = [rp.tile([P, Z], FP16, name=f"ring{r}") for r in range(RING)]
        act_scr = sp.tile([P, Z], FP16, name="act_scr")
        sq_scr = [sp.tile([P, Z], FP16, name=f"sq_scr{i}") for i in range(2)]
        ts_scr = [sp.tile([P, NUM_NEG], F32, name=f"ts{i}") for i in range(2)]
        ones_n = sp.tile([P, NUM_NEG], F32, name="ones_n")
        ancs = [apool.tile([P, Z], F32, name=f"anc{t}") for t in range(NT)]
        d2 = [sp.tile([P, NJ], F32, name=f"d2_{t}") for t in range(NT)]
        dan = [sp.tile([P, NUM_NEG], F32, name=f"dan{t}") for t in range(NT)]
        dap = [sp.tile([P, NUM_POS], F32, name=f"dap{t}") for t in range(NT)]
        lp = [sp.tile([P, NUM_POS], F32, name=f"lp{t}") for t in range(NT)]

        v.memset(ones_n[:, :], 1.0)

        # anc0 first on the sync queue (ahead of all chunks); anc1 via the
        # scalar (ACT) queue - ACT is idle until the first diff lands.
        nc.sync.dma_start(ancs[0][:, :], anc[0:P, :])
        nc.scalar.dma_start(ancs[1][:, :], anc[P : 2 * P, :])

        # all chunk DMAs, in stream order, on the sync queue
        tiles = {}
        for t in range(NT):
            for jj0, nsl in CHUNKS:
                xt = xp.tile([P, 2 * Z], F32, name="xt")
                nc.sync.dma_start(xt[:, 0 : nsl * Z], chunk_src(t, jj0, nsl))
                tiles[(t, jj0)] = xt

        # map slice -> (chunk_start, offset-within-chunk)
        CHUNK_OF = {}
        for jj0, nsl in CHUNKS:
            for q in range(nsl):
                CHUNK_OF[jj0 + q] = (jj0, q)

        def diff_op(t, jj, r):
            # ring[r] = (x - eps) - anc  (== -(anc + eps - x); squared later)
            jj0, q = CHUNK_OF[jj]
            xt = tiles[(t, jj0)]
            v.scalar_tensor_tensor(
                out=ring[r][:, :],
                in0=xt[:, q * Z : (q + 1) * Z],
                scalar=EPS,
                in1=ancs[t][:, :],
                op0=OP.subtract,
                op1=OP.subtract,
            )

        def act_sum(t, jj, r):
            act.activation(
                act_scr[:, :], ring[r][:, :], AF.Square,
                accum_out=d2[t][:, jj : jj + 1],
            )

        def dve_sum(t, jj, r, i):
            v.scalar_tensor_tensor(
                out=sq_scr[i % 2][:, :],
                in0=ring[r][:, :],
                scalar=1.0,
                in1=ring[r][:, :],
                op0=OP.bypass,
                op1=OP.mult,
                accum_out=d2[t][:, jj : jj + 1],
            )

        def sqrt_neg(t):
            act.activation(dan[t][:, :], d2[t][:, 0:NUM_NEG], AF.Sqrt)

        def sqrt_pos(t, p_i):
            act.activation(
                dap[t][:, p_i : p_i + 1],
                d2[t][:, NUM_NEG + p_i : NUM_NEG + p_i + 1],
                AF.Sqrt,
            )

        def pairing(t, p_i):
            # lp[:,p] = sum_n min(d_an - d_ap[p], 1); hinge = 16 - lp on host
            v.scalar_tensor_tensor(
                out=ts_scr[p_i % 2][:, :],
                in0=dan[t][:, :],
                scalar=dap[t][:, p_i : p_i + 1],
                op0=OP.subtract,
                in1=ones_n[:, :],
                op1=OP.min,
                accum_out=lp[t][:, p_i : p_i + 1],
            )

        ridx = 0
        rs = {}  # (t, jj) -> ring slot

        def do_diff(t, jj):
            nonlocal ridx
            r = ridx % RING
            ridx += 1
            rs[(t, jj)] = r
            diff_op(t, jj, r)

        # ---- tile 0: all 24 diffs (DVE); ACT sums follow per slice ----
        for jj in range(NJ):
            do_diff(0, jj)
            act_sum(0, jj, rs[(0, jj)])
            if jj == NUM_NEG - 1:
                sqrt_neg(0)
            if jj >= NUM_NEG:
                sqrt_pos(0, jj - NUM_NEG)

        # ---- tile 1 diffs, with tile-0 pairing inserted mid-stream ----
        for jj in range(PAIR_T0_AT):
            do_diff(1, jj)
            act_sum(1, jj, rs[(1, jj)])
        for p_i in range(NUM_POS):
            pairing(0, p_i)
        for jj in range(PAIR_T0_AT, NJ):
            do_diff(1, jj)
            if jj >= NJ - K_DVE:
                dve_sum(1, jj, rs[(1, jj)], jj)
            else:
                act_sum(1, jj, rs[(1, jj)])
            if jj == NUM_NEG - 1:
                sqrt_neg(1)
            if jj >= NUM_NEG:
                sqrt_pos(1, jj - NUM_NEG)
        for p_i in range(NUM_POS):
            pairing(1, p_i)
        # out triggers at the tail of the sync queue (it is idle by then;
        # the trigger waits on the lp sems without blocking anything else)
        nc.sync.dma_start(out[:, 0:NUM_POS], lp[0][:, :])
        nc.sync.dma_start(out[:, NUM_POS : 2 * NUM_POS], lp[1][:, :])


_NC_CACHE = None


def build():
    global _NC_CACHE
    if _NC_CACHE is None:
        nc = bacc.Bacc(
            "TRN2", target_bir_lowering=False, debug=False, num_devices=N_CORES
        )
        anc = nc.dram_tensor("anc", (BL, Z), F32, kind="ExternalInput").ap()
        pos = nc.dram_tensor("pos", (BL * NUM_POS, Z), F32, kind="ExternalInput").ap()
        neg = nc.dram_tensor("neg", (BL * NUM_NEG, Z), F32, kind="ExternalInput").ap()
        out = nc.dram_tensor("out", (P, NT * NUM_POS), F32, kind="ExternalOutput").ap()
        with tile.TileContext(nc) as tc:
            _emit(tc, nc, anc, pos, neg, out)
        nc.compile()
        _NC_CACHE = nc
    return _NC_CACHE


def make_in_maps(anc_embedding, pos_embedding, neg_embedding):
    anc_embedding = np.asarray(anc_embedding, dtype=np.float32)
    pos_embedding = np.asarray(pos_embedding, dtype=np.float32)
    neg_embedding = np.asarray(neg_embedding, dtype=np.float32)
    in_maps = []
    for c in range(N_CORES):
        in_maps.append(
            {
                "anc": np.ascontiguousarray(anc_embedding[c * BL : (c + 1) * BL]),
                "pos": np.ascontiguousarray(
                    pos_embedding[c * BL * NUM_POS : (c + 1) * BL * NUM_POS]
                ),
                "neg": np.ascontiguousarray(
                    neg_embedding[c * BL * NUM_NEG : (c + 1) * BL * NUM_NEG]
                ),
            }
        )
    return in_maps


def combine(outs):
    # each lp entry holds sum_n min(d_an - d_ap, 1) for one (row, pos) pair;
    # hinge sum for that pair = NUM_NEG - lp.  loss = mean over all pairs/negs.
    total_pairs = B * NUM_POS
    s = 0.0
    for o in outs:
        s += o.astype(np.float64).sum()
    return np.float32((NUM_NEG * total_pairs - s) / (B * NUM_POS * NUM_NEG))


def kernel(anc_embedding, pos_embedding, neg_embedding):
    nc = build()
    in_maps = make_in_maps(anc_embedding, pos_embedding, neg_embedding)
    res = bass_utils.run_bass_kernel_spmd(nc, in_maps, core_ids=list(range(N_CORES)))
    return combine([r["out"] for r in res.results])


# revision 14
# speedup vs baseline: 1.0117x; 1.0117x over previous
# Triplet-margin loss kernel for Trainium2 (Bass/Tile), batch-sharded
# across 8 NeuronCores.
#
# reference math (torch F.pairwise_distance semantics):
#   d_ap[b,p] = || anc[b] - pos[b,p] + eps ||_2
#   d_an[b,n] = || anc[b] - neg[b,n] + eps ||_2
#   loss = mean_{b,p,n} max(d_ap[b,p] - d_an[b,n] + margin, 0)
#
# v3 design (measured op costs on HW):
#   Three engines share the 48 slices' work:
#     DVE  stt diff (x - eps) - a -> fp16 ring   ~1143ns  (alternating bufs)
#     GpSimd tensor_tensor diff x - aprime       ~3300ns  (4 slices/tile;
#       no measurable DVE/ACT interference at this density)
#     ACT  Square(diff)+accum -> d2 col          ~1366ns
#     DVE  stt bypass/mult+accum sum (d2 col)    ~1267ns  (for the gpsimd
#       slices and the last two slices of tile 1)
#   tensor_tensor_reduce is NOT used: it crashes the exec unit on this HW.
#   Slice order per tile is neg0..15 then pos0..7; sqrt(neg) [128,16] runs
#   mid-tile, sqrt(pos) [128,8] once per tile (per-column sqrts cost 294ns
#   each on ACT - too much), then 8 pairing ops on DVE:
#     lp[:,p] = sum_n min(d_an - d_ap[p], 1);  hinge = 16 - lp on host.
#   ACT's table is primed with tiny Square+Sqrt ops at kernel start so the
#   1.3us ACT_TABLE_LOAD happens while ACT idles in the DMA ramp instead
#   of stalling the sum stream.
#   anc0 is the FIRST transfer on the sync queue (everything needs it);
#   anc1 rides the same queue after tile-0's chunks.  aprime (= anc + eps,
#   for the gpsimd diffs only) is computed by ACT during its idle ramp.
#   The 14 chunk buffers are shared between the two batch-tiles (tile-1's
#   chunk c reuses tile-0's buffer c; WAR handled by the tile framework),
#   halving the tile-release count in the fixed teardown.
#   Tile-0 pairing is emitted mid-way through tile-1's diff stream.

import numpy as np

import concourse.bacc as bacc
import concourse.mybir as mybir
import concourse.tile as tile
from concourse import bass_utils

B, Z = 2048, 1024
NUM_POS, NUM_NEG = 8, 16
NJ = NUM_POS + NUM_NEG
MARGIN, EPS = 1.0, 1e-6
N_CORES = 8
BL = B // N_CORES  # 256 rows of anc per core
P = 128
NT = BL // P  # 2 batch-tiles per core
RING = 16
N_SINGLES = 4  # lead single-slice chunks per tile
PAIR_T0_AT = 9  # tile-0 pairing emitted after this many tile-1 diffs

GP_DIFF = (3, 9, 15, 19)  # slices whose diff runs on gpsimd (per tile)
DVE_SUM = {0: set(GP_DIFF), 1: set(GP_DIFF) | {22, 23}}  # sums on DVE

F32 = mybir.dt.float32
FP16 = mybir.dt.float16
AF = mybir.ActivationFunctionType
OP = mybir.AluOpType

# chunk list: (first_slice, n_slices) in the neg-first slice order
CHUNKS = (
    [(j, 1) for j in range(N_SINGLES)]
    + [(j, 2) for j in range(N_SINGLES, NJ, 2)]
)
NCH = len(CHUNKS)


def _emit(tc, nc, anc, pos, neg, out):
    v = nc.vector
    act = nc.scalar
    gp = nc.gpsimd
    pos2 = pos.rearrange("(b j) z -> b (j z)", j=NUM_POS)  # [BL, 8*Z]
    neg2 = neg.rearrange("(b j) z -> b (j z)", j=NUM_NEG)  # [BL, 16*Z]

    def chunk_src(t, jj0, nsl):
        b0 = t * P
        if jj0 < NUM_NEG:
            return neg2[b0 : b0 + P, jj0 * Z : (jj0 + nsl) * Z]
        return pos2[b0 : b0 + P, (jj0 - NUM_NEG) * Z : (jj0 - NUM_NEG + nsl) * Z]

    CHUNK_OF = {}  # slice -> (chunk index, offset-within-chunk)
    for c, (jj0, nsl) in enumerate(CHUNKS):
        for q in range(nsl):
            CHUNK_OF[jj0 + q] = (c, q)

    with (
        tc.tile_pool(name="xp", bufs=1) as xp,
        tc.tile_pool(name="rp", bufs=1) as rp,
        tc.tile_pool(name="sp", bufs=1) as sp,
    ):
        xt = [xp.tile([P, 2 * Z], F32, name=f"xt{c}") for c in range(NCH)]
        ring = [rp.tile([P, Z], FP16, name=f"ring{r}") for r in range(RING)]
        act_scr = sp.tile([P, Z], FP16, name="act_scr")
        sq_scr = [sp.tile([P, Z], FP16, name=f"sq_scr{i}") for i in range(2)]
        ts_scr = [sp.tile([P, NUM_NEG], F32, name=f"ts{i}") for i in range(2)]
        ones_n = sp.tile([P, NUM_NEG], F32, name="ones_n")
        eps_t = sp.tile([P, 1], F32, name="eps_t")
        prime = sp.tile([P, 2], F32, name="prime")
        ancs = [sp.tile([P, Z], F32, name=f"anc{t}") for t in range(NT)]
        aprime = [sp.tile([P, Z], F32, name=f"aprime{t}") for t in range(NT)]
        d2 = [sp.tile([P, NJ], F32, name=f"d2_{t}") for t in range(NT)]
        dan = [sp.tile([P, NUM_NEG], F32, name=f"dan{t}") for t in range(NT)]
        dap = [sp.tile([P, NUM_POS], F32, name=f"dap{t}") for t in range(NT)]
        lp = [sp.tile([P, NUM_POS], F32, name=f"lp{t}") for t in range(NT)]

        v.memset(ones_n[:, :], 1.0)
        v.memset(eps_t[:, :], EPS)

        # prime the ACT function table (Square+Sqrt) during the DMA ramp
        act.activation(prime[:, 0:1], ones_n[:, 0:1], AF.Square)
        act.activation(prime[:, 1:2], ones_n[:, 0:1], AF.Sqrt)

        # anc0 first on the sync queue, then tile-0's chunks.  (tile-1's
        # chunk DMAs are emitted AFTER tile-0's compute so the framework
        # sees the WAR on the shared xt buffers.)
        nc.sync.dma_start(ancs[0][:, :], anc[0:P, :])
        for c, (jj0, nsl) in enumerate(CHUNKS):
            nc.sync.dma_start(xt[c][:, 0 : nsl * Z], chunk_src(0, jj0, nsl))

        # aprime = anc + eps on ACT (idle during the ramp); gpsimd diffs use it
        act.activation(aprime[0][:, :], ancs[0][:, :], AF.Identity, bias=eps_t[:, 0:1])

        def dve_diff(t, jj, r):
            c, q = CHUNK_OF[jj]
            v.scalar_tensor_tensor(
                out=ring[r][:, :],
                in0=xt[c][:, q * Z : (q + 1) * Z],
                scalar=EPS,
                in1=ancs[t][:, :],
                op0=OP.subtract,
                op1=OP.subtract,
            )

        def gp_diff(t, jj, r):
            c, q = CHUNK_OF[jj]
            gp.tensor_tensor(
                out=ring[r][:, :],
                in0=xt[c][:, q * Z : (q + 1) * Z],
                in1=aprime[t][:, :],
                op=OP.subtract,
            )

        def act_sum(t, jj, r):
            act.activation(
                act_scr[:, :], ring[r][:, :], AF.Square,
                accum_out=d2[t][:, jj : jj + 1],
            )

        sqi = [0]

        def dve_sum(t, jj, r):
            v.scalar_tensor_tensor(
                out=sq_scr[sqi[0] % 2][:, :],
                in0=ring[r][:, :],
                scalar=1.0,
                in1=ring[r][:, :],
                op0=OP.bypass,
                op1=OP.mult,
                accum_out=d2[t][:, jj : jj + 1],
            )
            sqi[0] += 1

        def sqrt_neg(t):
            act.activation(dan[t][:, :], d2[t][:, 0:NUM_NEG], AF.Sqrt)

        def sqrt_pos(t):
            act.activation(dap[t][:, :], d2[t][:, NUM_NEG:NJ], AF.Sqrt)

        def pairing(t, p_i):
            # lp[:,p] = sum_n min(d_an - d_ap[p], 1); hinge = 16 - lp on host
            v.scalar_tensor_tensor(
                out=ts_scr[p_i % 2][:, :],
                in0=dan[t][:, :],
                scalar=dap[t][:, p_i : p_i + 1],
                op0=OP.subtract,
                in1=ones_n[:, :],
                op1=OP.min,
                accum_out=lp[t][:, p_i : p_i + 1],
            )

        slot = {}
        nxt = [0]

        def assign_slot(t, jj):
            r = nxt[0] % RING
            nxt[0] += 1
            slot[(t, jj)] = r
            return r

        # DVE-sum slices queue up; DVE issues a pending sum after each diff
        pending = []

        def flush_one_sum():
            if pending:
                t_, jj_ = pending.pop(0)
                dve_sum(t_, jj_, slot[(t_, jj_)])

        def do_slice(t, jj):
            r = assign_slot(t, jj)
            if jj in GP_DIFF:
                gp_diff(t, jj, r)
            else:
                dve_diff(t, jj, r)
            if jj in DVE_SUM[t]:
                pending.append((t, jj))
            else:
                act_sum(t, jj, r)
            if jj not in GP_DIFF:
                flush_one_sum()
            # all 16 neg sums are emitted once slice 16's flush has run
            # (slice 15's sum is DVE-pending until then)
            if jj == NUM_NEG:
                sqrt_neg(t)

        # ---- tile 0 ----
        for jj in range(NJ):
            do_slice(0, jj)
        sqrt_pos(0)

        # tile-1 DMAs (after tile-0 compute emission: WAR on shared xt)
        nc.sync.dma_start(ancs[1][:, :], anc[P : 2 * P, :])
        for c, (jj0, nsl) in enumerate(CHUNKS):
            nc.sync.dma_start(xt[c][:, 0 : nsl * Z], chunk_src(1, jj0, nsl))
        act.activation(aprime[1][:, :], ancs[1][:, :], AF.Identity, bias=eps_t[:, 0:1])

        # ---- tile 1 diffs, with tile-0 pairing inserted mid-stream ----
        for jj in range(PAIR_T0_AT):
            do_slice(1, jj)
        for p_i in range(NUM_POS):
            pairing(0, p_i)
        for jj in range(PAIR_T0_AT, NJ):
            do_slice(1, jj)
        while pending:
            flush_one_sum()
        sqrt_pos(1)
        for p_i in range(NUM_POS):
            pairing(1, p_i)
        nc.sync.dma_start(out[:, 0:NUM_POS], lp[0][:, :])
        nc.sync.dma_start(out[:, NUM_POS : 2 * NUM_POS], lp[1][:, :])


_NC_CACHE = None


def build():
    global _NC_CACHE
    if _NC_CACHE is None:
        nc = bacc.Bacc(
            "TRN2", target_bir_lowering=False, debug=False, num_devices=N_CORES
        )
        anc = nc.dram_tensor("anc", (BL, Z), F32, kind="ExternalInput").ap()
        pos = nc.dram_tensor("pos", (BL * NUM_POS, Z), F32, kind="ExternalInput").ap()
        neg = nc.dram_tensor("neg", (BL * NUM_NEG, Z), F32, kind="ExternalInput").ap()
        out = nc.dram_tensor("out", (P, NT * NUM_POS), F32, kind="ExternalOutput").ap()
        with tile.TileContext(nc) as tc:
            _emit(tc, nc, anc, pos, neg, out)
        nc.compile()
        _NC_CACHE = nc
    return _NC_CACHE


def make_in_maps(anc_embedding, pos_embedding, neg_embedding):
    anc_embedding = np.asarray(anc_embedding, dtype=np.float32)
    pos_embedding = np.asarray(pos_embedding, dtype=np.float32)
    neg_embedding = np.asarray(neg_embedding, dtype=np.float32)
    in_maps = []
    for c in range(N_CORES):
        in_maps.append(
            {
                "anc": np.ascontiguousarray(anc_embedding[c * BL : (c + 1) * BL]),
                "pos": np.ascontiguousarray(
                    pos_embedding[c * BL * NUM_POS : (c + 1) * BL * NUM_POS]
                ),
                "neg": np.ascontiguousarray(
                    neg_embedding[c * BL * NUM_NEG : (c + 1) * BL * NUM_NEG]
                ),
            }
        )
    return in_maps


def combine(outs):
    # each lp entry holds sum_n min(d_an - d_ap, 1) for one (row, pos) pair;
    # hinge sum for that pair = NUM_NEG - lp.  loss = mean over all pairs/negs.
    total_pairs = B * NUM_POS
    s = 0.0
    for o in outs:
        s += o.astype(np.float64).sum()
    return np.float32((NUM_NEG * total_pairs - s) / (B * NUM_POS * NUM_NEG))


def kernel(anc_embedding, pos_embedding, neg_embedding):
    nc = build()
    in_maps = make_in_maps(anc_embedding, pos_embedding, neg_embedding)
    res = bass_utils.run_bass_kernel_spmd(nc, in_maps, core_ids=list(range(N_CORES)))
    return combine([r["out"] for r in res.results])
